# revision 43
# baseline (speedup 1.0000x reference)
import sys

for p in ("/opt/trn_rl_repo",):
    if p not in sys.path:
        sys.path.insert(0, p)

import numpy as np
import ml_dtypes

import concourse.bass as bass
from concourse import bacc
import concourse.mybir as mybir
import concourse.tile as tile
from concourse.bass import ds, ts
from concourse.bass_utils import run_bass_kernel_spmd

BF16 = ml_dtypes.bfloat16

B, N, DIM, NH = 256, 196, 256, 8
HD = DIM // NH  # 32
G = 14
NCORES = 8
BLOC = B // NCORES  # 32
NC2 = 98  # N / 2
BLK = 4  # batch block per pipeline stage

# QK psum supertile [98, 4, 2, 256] f32 = 4 banks per (b, group): local head l
# owns bank l (so each PE row-tile writes exactly one bank — row tiles must
# never share a bank), with the two m-chunks j at 1 KiB offsets inside it.


def _relative_position_index(g: int) -> np.ndarray:
    coords = np.stack(np.meshgrid(np.arange(g), np.arange(g), indexing="ij"))
    cf = coords.reshape(2, -1)
    rel = cf[:, :, None] - cf[:, None, :]
    rel = rel.transpose(1, 2, 0).astype(np.int64)
    rel[..., 0] += g - 1
    rel[..., 1] += g - 1
    rel[..., 0] *= 2 * g - 1
    return rel.sum(-1)


def _bias_coords(g: int) -> np.ndarray:
    p = np.arange(1 - g, g)
    biases = np.stack(np.meshgrid(p, p, indexing="ij"))
    return biases.reshape(2, -1).T.astype(np.float32)


_CACHED = {}


def _build_bass():
    if "nc" in _CACHED:
        return _CACHED["nc"]
    f32 = mybir.dt.float32
    bf16 = mybir.dt.bfloat16

    nc = bacc.Bacc("TRN2", target_bir_lowering=False)
    qt_d = nc.dram_tensor("qt", [2, 128, BLOC, 196], bf16, kind="ExternalInput")
    kt_d = nc.dram_tensor("kt", [2, 128, BLOC, 196], bf16, kind="ExternalInput")
    v_d = nc.dram_tensor("v", [NC2, BLOC, 2, 8, 32], bf16, kind="ExternalInput")
    erpb_d = nc.dram_tensor("erpb", [NC2, 2, 2, 4, 2, 196], bf16, kind="ExternalInput")
    w_d = nc.dram_tensor("w", [128, 2, 256], bf16, kind="ExternalInput")
    pb_d = nc.dram_tensor("pb", [NC2, 256], bf16, kind="ExternalInput")
    out_d = nc.dram_tensor("out", [BLOC, 196, 256], bf16, kind="ExternalOutput")

    from contextlib import ExitStack

    with tile.TileContext(nc) as tc, ExitStack() as es:
        const = es.enter_context(tc.tile_pool(name="const", bufs=1))
        io = es.enter_context(tc.tile_pool(name="io", bufs=2))
        work = es.enter_context(tc.tile_pool(name="work", bufs=2))
        psum_qk = es.enter_context(tc.tile_pool(name="psum_qk", bufs=2, space="PSUM"))
        psum_pv = es.enter_context(tc.tile_pool(name="psum_pv", bufs=1, space="PSUM"))
        psum_po = es.enter_context(tc.tile_pool(name="psum_po", bufs=2, space="PSUM"))

        erpb_sb = const.tile([NC2, 2, 2, 4, 2, 196], bf16)
        nc.sync.dma_start(erpb_sb[:], erpb_d[:])
        w_sb = const.tile([128, 2, 256], bf16)
        nc.sync.dma_start(w_sb[:], w_d[:])
        pb_sb = const.tile([NC2, 256], bf16)
        nc.sync.dma_start(pb_sb[:], pb_d[:])
        ones32 = const.tile([NC2, 32], bf16)
        nc.vector.memset(ones32[:], 1.0)
        # preload the exp table set during the initial DMA wait
        warm = const.tile([1, 8], f32)
        nc.scalar.activation(warm[:], warm[:], mybir.ActivationFunctionType.Exp)

        def tail_pv_g(pv, v_sb, pst, b4, g):
            # pv slots: [xT_g0, xT_g1, den_g0, den_g1] → xT in bank 0,
            # denominators (replicated via 32-wide ones weights) in bank 1
            for l in range(4):
                for j in range(2):
                    nc.tensor.matmul(
                        pv[ds(32 * l, 32), g, 0:196],
                        lhsT=v_sb[:, b4, j, 4 * g + l],
                        rhs=pst[g][:, b4, l, j],
                        start=(j == 0),
                        stop=(j == 1),
                        tile_position=(0, 32 * l),
                    )
            for l in range(4):
                for j in range(2):
                    nc.tensor.matmul(
                        pv[ds(32 * l, 32), 2 + g, 0:196],
                        lhsT=ones32[:],
                        rhs=pst[g][:, b4, l, j],
                        start=(j == 0),
                        stop=(j == 1),
                        tile_position=(0, 32 * l),
                    )

        def tail_norm(pv, bb, b4):
            rcp = work.tile([128, 2, 196], f32, tag="rcp", name=f"rcp_{bb}_{b4}")
            nc.vector.reciprocal_approx_fast(rcp[:], pv[:, 2:4, 0:196])
            xnt = work.tile(
                [128, 2, 196], bf16, tag="xnt", name=f"xnt_{bb}_{b4}", bufs=4
            )
            nc.vector.tensor_mul(out=xnt[:], in0=pv[:, 0:2, 0:196], in1=rcp[:])
            return xnt

        def tail_proj(bb, b4, xnt):
            po = psum_po.tile([NC2, 2, 256], f32, tag="po", name=f"po_{bb}_{b4}")
            for i in range(2):
                for g in range(2):
                    nc.tensor.matmul(
                        po[:, i],
                        lhsT=xnt[:, g, ds(98 * i, 98)],
                        rhs=w_sb[:, g],
                        start=(g == 0),
                        stop=(g == 1),
                    )
            o_sb = work.tile([NC2, 2, 256], bf16, tag="o", name=f"o_{bb}_{b4}")
            nc.vector.tensor_add(
                out=o_sb[:],
                in0=po[:],
                in1=pb_sb[:, None].to_broadcast([NC2, 2, 256]),
            )
            nc.sync.dma_start(
                out_d[bb + b4].rearrange("(i p) c -> p i c", p=NC2), o_sb[:]
            )

        def load_block(bb):
            qt_sb = [
                io.tile([128, BLK, 196], bf16, tag=f"qt{g}", name=f"qt{g}_{bb}")
                for g in range(2)
            ]
            kt_sb = [
                io.tile([128, BLK, 196], bf16, tag=f"kt{g}", name=f"kt{g}_{bb}")
                for g in range(2)
            ]
            for g in range(2):
                nc.sync.dma_start(qt_sb[g][:], qt_d[g, :, ds(bb, BLK)])
                nc.sync.dma_start(kt_sb[g][:], kt_d[g, :, ds(bb, BLK)])
            v_sb = io.tile(
                [NC2, BLK, 2, 8, 32], bf16, tag="v", name=f"v_{bb}", bufs=3
            )
            nc.sync.dma_start(v_sb[:], v_d[:, ds(bb, BLK)])
            return qt_sb, kt_sb, v_sb

        prev = None
        loaded = load_block(0)
        for bb in range(0, BLOC, BLK):
            qt_sb, kt_sb, v_sb = loaded
            if bb + BLK < BLOC:
                loaded = load_block(bb + BLK)

            est = {}
            pst = {}
            for g in range(2):
                est[g] = work.tile(
                    [NC2, BLK, 4, 2, 196], bf16, tag=f"est{g}", name=f"est{g}_{bb}"
                )
                pst[g] = work.tile(
                    [NC2, BLK, 4, 2, 196], bf16, tag=f"pst{g}", name=f"pst{g}_{bb}"
                )

            def qk_exp(g, b4):
                # per (b, g, head-pair) supertile [98, 2, 2, 256] f32 = 2
                # banks: each head's PE row-tile owns one full psum bank;
                # 2-bank halves double-buffer so ACT never waits on refill
                for hp in range(2):
                    sqk = psum_qk.tile(
                        [NC2, 2, 2, 256], f32, tag="qk", name=f"qk{g}{hp}_{bb}_{b4}"
                    )
                    for j in range(2):
                        for dl in range(2):
                            l = 2 * hp + dl
                            nc.tensor.matmul(
                                sqk[:, dl, j, 0:196],
                                lhsT=kt_sb[g][ds(32 * l, 32), b4, ds(98 * j, 98)],
                                rhs=qt_sb[g][ds(32 * l, 32), b4],
                                start=True,
                                stop=True,
                                tile_position=(32 * l, 0),
                            )
                    nc.scalar.activation(
                        est[g][:, b4, ds(2 * hp, 2)],
                        sqk[:, :, :, 0:196],
                        mybir.ActivationFunctionType.Exp,
                    )

            def mul(g, h2):
                nc.vector.tensor_mul(
                    out=pst[g][:, ds(2 * h2, 2)],
                    in0=est[g][:, ds(2 * h2, 2)],
                    in1=erpb_sb[:, g],
                )

            # software-pipelined issue: exp stages of block i interleaved with
            # PV/norm of block i-1 and late proj units of block i-2, so the
            # PE FIFO never holds ACT hostage at block seams
            pb4 = prev
            xnts = [None] * BLK

            def t_unit(b4):
                if not pb4:
                    return
                pv = psum_pv.tile(
                    [128, 4, 256], f32, tag="pv", name=f"pv_{pb4[0]}_{b4}"
                )
                tail_pv_g(pv, pb4[1], pb4[2], b4, 0)
                tail_pv_g(pv, pb4[1], pb4[2], b4, 1)
                xnts[b4] = tail_norm(pv, pb4[0], b4)

            qk_exp(0, 0)
            qk_exp(0, 1)
            qk_exp(0, 2)
            t_unit(0)
            mul(0, 0)
            qk_exp(0, 3)
            t_unit(1)
            mul(0, 1)
            qk_exp(1, 0)
            t_unit(2)
            qk_exp(1, 1)
            t_unit(3)
            mul(1, 0)
            qk_exp(1, 2)
            if pb4:
                tail_proj(pb4[0], 0, xnts[0])
                tail_proj(pb4[0], 1, xnts[1])
            qk_exp(1, 3)
            mul(1, 1)
            if pb4:
                tail_proj(pb4[0], 2, xnts[2])
                tail_proj(pb4[0], 3, xnts[3])

            prev = (bb, v_sb, pst)

        for b4 in range(BLK):
            pv = psum_pv.tile([128, 4, 256], f32, tag="pv", name=f"pvf_{b4}")
            tail_pv_g(pv, prev[1], prev[2], b4, 0)
            tail_pv_g(pv, prev[1], prev[2], b4, 1)
            xnt = tail_norm(pv, prev[0], b4)
            tail_proj(prev[0], b4, xnt)

    nc.compile()
    _CACHED["nc"] = nc
    return nc


def _prep_host(q, k, v, dpb_w1, dpb_b1, dpb_w2, dpb_b2, proj_w, proj_b):
    scale = HD ** -0.5
    # qT/kT [2, 128, B, 196]: [g, 32*l + d, b, n] = q[b, n, (4g+l)*32 + d]
    qs = (q.astype(np.float32) * scale).transpose(2, 0, 1).reshape(2, 128, B, N)
    qt = np.ascontiguousarray(qs).astype(BF16)
    ks = k.astype(np.float32).transpose(2, 0, 1).reshape(2, 128, B, N)
    kt = np.ascontiguousarray(ks).astype(BF16)
    # v [98, B, 2, 8, 32]: [p, b, j, h, d] = v[b, 98j+p, 32h+d]
    vr = v.reshape(B, 2, NC2, NH, HD).transpose(2, 0, 1, 3, 4)
    vx = np.ascontiguousarray(vr).astype(BF16)
    # rpb via MLP on host
    biases = _bias_coords(G)
    pos = np.maximum(biases @ dpb_w1 + dpb_b1, 0.0) @ dpb_w2 + dpb_b2  # [729, 8]
    idx = _relative_position_index(G).reshape(-1)
    rpb = pos[idx].reshape(N, N, NH).transpose(2, 0, 1)  # [H, n, m]
    # erpb [98, 2, 2, 4, 2, 196]: [p, g, brep, l, j, n] = exp(rpb[4g+l, n, 98j+p])
    # (replicated over a 2-wide b axis so the DVE multiply reads contiguous
    # stride-1 data and hits the 2x bf16 perf mode)
    er = np.exp(rpb)  # [h, n, m]
    erpb1 = np.empty((NC2, 2, 4, 2, N), np.float32)
    for g in range(2):
        for l in range(4):
            for j in range(2):
                erpb1[:, g, l, j, :] = er[4 * g + l, :, 98 * j : 98 * j + 98].T
    erpb = np.ascontiguousarray(
        np.broadcast_to(erpb1[:, :, None], (NC2, 2, 2, 4, 2, N))
    ).astype(BF16)
    # w [128, 2, 256]: [32l+d, g, co] = proj_w[(4g+l)*32 + d, co]
    w = np.ascontiguousarray(
        proj_w.reshape(2, 128, 256).transpose(1, 0, 2)
    ).astype(BF16)
    pb = np.broadcast_to(proj_b.reshape(1, 256), (NC2, 256))
    pb = np.ascontiguousarray(pb).astype(BF16)
    return qt, kt, vx, erpb, w, pb


def kernel(**inputs) -> np.ndarray:
    q = np.asarray(inputs["q"], np.float32)
    k = np.asarray(inputs["k"], np.float32)
    v = np.asarray(inputs["v"], np.float32)
    qt, kt, vx, erpb, w, pb = _prep_host(
        q, k, v,
        np.asarray(inputs["dpb_w1"], np.float32),
        np.asarray(inputs["dpb_b1"], np.float32),
        np.asarray(inputs["dpb_w2"], np.float32),
        np.asarray(inputs["dpb_b2"], np.float32),
        np.asarray(inputs["proj_w"], np.float32),
        np.asarray(inputs["proj_b"], np.float32),
    )
    nc = _build_bass()
    in_maps = []
    for c in range(NCORES):
        sl = slice(c * BLOC, (c + 1) * BLOC)
        in_maps.append(
            {
                "qt": np.ascontiguousarray(qt[:, :, sl]),
                "kt": np.ascontiguousarray(kt[:, :, sl]),
                "v": np.ascontiguousarray(vx[:, sl]),
                "erpb": erpb,
                "w": w,
                "pb": pb,
            }
        )
    res = run_bass_kernel_spmd(
        nc, in_maps, core_ids=list(range(NCORES)), trace=bool(_CACHED.get("trace"))
    )
    _CACHED["last_results"] = res
    out = np.concatenate([r["out"] for r in res.results], axis=0)
    return out.astype(np.float32)


if __name__ == "__main__":
    rng = np.random.default_rng(0)
    ins = {
        "q": rng.standard_normal((B, N, DIM), dtype=np.float32),
        "k": rng.standard_normal((B, N, DIM), dtype=np.float32),
        "v": rng.standard_normal((B, N, DIM), dtype=np.float32),
        "dpb_w1": rng.standard_normal((2, 64), dtype=np.float32) * 0.1,
        "dpb_b1": np.zeros(64, np.float32),
        "dpb_w2": rng.standard_normal((64, 8), dtype=np.float32) * 0.1,
        "dpb_b2": np.zeros(8, np.float32),
        "proj_w": rng.standard_normal((256, 256), dtype=np.float32) * (256 ** -0.5),
        "proj_b": np.zeros(256, np.float32),
        "group_size": 14,
    }
    o = kernel(**ins)
    print(o.shape, o.dtype)


# revision 44
# speedup vs baseline: 1.0115x; 1.0115x over previous
import sys

for p in ("/opt/trn_rl_repo",):
    if p not in sys.path:
        sys.path.insert(0, p)

import numpy as np
import ml_dtypes

import concourse.bass as bass
from concourse import bacc
import concourse.mybir as mybir
import concourse.tile as tile
from concourse.bass import ds, ts
from concourse.bass_utils import run_bass_kernel_spmd

BF16 = ml_dtypes.bfloat16

B, N, DIM, NH = 256, 196, 256, 8
HD = DIM // NH  # 32
G = 14
NCORES = 8
BLOC = B // NCORES  # 32
NC2 = 98  # N / 2
BLK = 4  # batch block per pipeline stage

# QK psum supertile [98, 4, 2, 256] f32 = 4 banks per (b, group): local head l
# owns bank l (so each PE row-tile writes exactly one bank — row tiles must
# never share a bank), with the two m-chunks j at 1 KiB offsets inside it.


def _relative_position_index(g: int) -> np.ndarray:
    coords = np.stack(np.meshgrid(np.arange(g), np.arange(g), indexing="ij"))
    cf = coords.reshape(2, -1)
    rel = cf[:, :, None] - cf[:, None, :]
    rel = rel.transpose(1, 2, 0).astype(np.int64)
    rel[..., 0] += g - 1
    rel[..., 1] += g - 1
    rel[..., 0] *= 2 * g - 1
    return rel.sum(-1)


def _bias_coords(g: int) -> np.ndarray:
    p = np.arange(1 - g, g)
    biases = np.stack(np.meshgrid(p, p, indexing="ij"))
    return biases.reshape(2, -1).T.astype(np.float32)


_CACHED = {}


def _build_bass():
    if "nc" in _CACHED:
        return _CACHED["nc"]
    f32 = mybir.dt.float32
    bf16 = mybir.dt.bfloat16

    nc = bacc.Bacc("TRN2", target_bir_lowering=False)
    qt_d = nc.dram_tensor("qt", [2, 128, BLOC, 196], bf16, kind="ExternalInput")
    kt_d = nc.dram_tensor("kt", [2, 128, BLOC, 196], bf16, kind="ExternalInput")
    v_d = nc.dram_tensor("v", [NC2, BLOC, 2, 8, 32], bf16, kind="ExternalInput")
    erpb_d = nc.dram_tensor("erpb", [NC2, 2, 2, 4, 2, 196], bf16, kind="ExternalInput")
    w_d = nc.dram_tensor("w", [128, 2, 256], bf16, kind="ExternalInput")
    pb_d = nc.dram_tensor("pb", [NC2, 256], bf16, kind="ExternalInput")
    out_d = nc.dram_tensor("out", [BLOC, 196, 256], bf16, kind="ExternalOutput")

    from contextlib import ExitStack

    with tile.TileContext(nc) as tc, ExitStack() as es:
        const = es.enter_context(tc.tile_pool(name="const", bufs=1))
        io = es.enter_context(tc.tile_pool(name="io", bufs=2))
        work = es.enter_context(tc.tile_pool(name="work", bufs=2))
        psum_qk = es.enter_context(tc.tile_pool(name="psum_qk", bufs=2, space="PSUM"))
        psum_pv = es.enter_context(tc.tile_pool(name="psum_pv", bufs=1, space="PSUM"))
        psum_po = es.enter_context(tc.tile_pool(name="psum_po", bufs=2, space="PSUM"))

        erpb_sb = const.tile([NC2, 2, 2, 4, 2, 196], bf16)
        nc.sync.dma_start(erpb_sb[:], erpb_d[:])
        w_sb = const.tile([128, 2, 256], bf16)
        nc.sync.dma_start(w_sb[:], w_d[:])
        pb_sb = const.tile([NC2, 256], bf16)
        nc.sync.dma_start(pb_sb[:], pb_d[:])
        ones32 = const.tile([NC2, 32], bf16)
        nc.vector.memset(ones32[:], 1.0)
        # preload the exp table set during the initial DMA wait
        warm = const.tile([1, 8], f32)
        nc.scalar.activation(warm[:], warm[:], mybir.ActivationFunctionType.Exp)

        def tail_pv_g(pv, v_sb, pst, b4, g):
            # pv slots: [xT_g0, xT_g1, den_g0, den_g1] → xT in bank 0,
            # denominators (replicated via 32-wide ones weights) in bank 1
            for l in range(4):
                for j in range(2):
                    nc.tensor.matmul(
                        pv[ds(32 * l, 32), g, 0:196],
                        lhsT=v_sb[:, b4, j, 4 * g + l],
                        rhs=pst[g][:, b4, l, j],
                        start=(j == 0),
                        stop=(j == 1),
                        tile_position=(0, 32 * l),
                    )
            for l in range(4):
                for j in range(2):
                    nc.tensor.matmul(
                        pv[ds(32 * l, 32), 2 + g, 0:196],
                        lhsT=ones32[:],
                        rhs=pst[g][:, b4, l, j],
                        start=(j == 0),
                        stop=(j == 1),
                        tile_position=(0, 32 * l),
                    )

        def tail_norm(pv, bb, b4):
            rcp = work.tile([128, 2, 196], f32, tag="rcp", name=f"rcp_{bb}_{b4}")
            nc.vector.reciprocal_approx_fast(rcp[:], pv[:, 2:4, 0:196])
            xnt = work.tile(
                [128, 2, 196], bf16, tag="xnt", name=f"xnt_{bb}_{b4}", bufs=4
            )
            nc.vector.tensor_mul(out=xnt[:], in0=pv[:, 0:2, 0:196], in1=rcp[:])
            return xnt

        def tail_proj(bb, b4, xnt):
            po = psum_po.tile([NC2, 2, 256], f32, tag="po", name=f"po_{bb}_{b4}")
            for i in range(2):
                for g in range(2):
                    nc.tensor.matmul(
                        po[:, i],
                        lhsT=xnt[:, g, ds(98 * i, 98)],
                        rhs=w_sb[:, g],
                        start=(g == 0),
                        stop=(g == 1),
                    )
            o_sb = work.tile([NC2, 2, 256], bf16, tag="o", name=f"o_{bb}_{b4}")
            nc.vector.tensor_add(
                out=o_sb[:],
                in0=po[:],
                in1=pb_sb[:, None].to_broadcast([NC2, 2, 256]),
            )
            nc.sync.dma_start(
                out_d[bb + b4].rearrange("(i p) c -> p i c", p=NC2), o_sb[:]
            )

        def load_block(bb):
            qt_sb = [
                io.tile([128, BLK, 196], bf16, tag=f"qt{g}", name=f"qt{g}_{bb}")
                for g in range(2)
            ]
            kt_sb = [
                io.tile([128, BLK, 196], bf16, tag=f"kt{g}", name=f"kt{g}_{bb}")
                for g in range(2)
            ]
            for g in range(2):
                nc.sync.dma_start(qt_sb[g][:], qt_d[g, :, ds(bb, BLK)])
                nc.sync.dma_start(kt_sb[g][:], kt_d[g, :, ds(bb, BLK)])
            v_sb = io.tile(
                [NC2, BLK, 2, 8, 32], bf16, tag="v", name=f"v_{bb}", bufs=3
            )
            nc.sync.dma_start(v_sb[:], v_d[:, ds(bb, BLK)])
            return qt_sb, kt_sb, v_sb

        prev = None
        loaded = load_block(0)
        for bb in range(0, BLOC, BLK):
            qt_sb, kt_sb, v_sb = loaded
            if bb + BLK < BLOC:
                loaded = load_block(bb + BLK)

            est = {}
            pst = {}
            for g in range(2):
                est[g] = work.tile(
                    [NC2, BLK, 4, 2, 196], bf16, tag=f"est{g}", name=f"est{g}_{bb}"
                )
                pst[g] = work.tile(
                    [NC2, BLK, 4, 2, 196], bf16, tag=f"pst{g}", name=f"pst{g}_{bb}"
                )

            def qk_exp(g, b4):
                # per (b, g, head-pair) supertile [98, 2, 2, 256] f32 = 2
                # banks: each head's PE row-tile owns one full psum bank;
                # 2-bank halves double-buffer so ACT never waits on refill
                for hp in range(2):
                    sqk = psum_qk.tile(
                        [NC2, 2, 2, 256], f32, tag="qk", name=f"qk{g}{hp}_{bb}_{b4}"
                    )
                    for j in range(2):
                        for dl in range(2):
                            l = 2 * hp + dl
                            nc.tensor.matmul(
                                sqk[:, dl, j, 0:196],
                                lhsT=kt_sb[g][ds(32 * l, 32), b4, ds(98 * j, 98)],
                                rhs=qt_sb[g][ds(32 * l, 32), b4],
                                start=True,
                                stop=True,
                                tile_position=(32 * l, 0),
                            )
                    nc.scalar.activation(
                        est[g][:, b4, ds(2 * hp, 2)],
                        sqk[:, :, :, 0:196],
                        mybir.ActivationFunctionType.Exp,
                    )

            def mul(g, h2):
                nc.vector.tensor_mul(
                    out=pst[g][:, ds(2 * h2, 2)],
                    in0=est[g][:, ds(2 * h2, 2)],
                    in1=erpb_sb[:, g],
                )

            # software-pipelined issue: exp stages of block i interleaved with
            # PV/norm of block i-1 and late proj units of block i-2, so the
            # PE FIFO never holds ACT hostage at block seams
            pb4 = prev
            xnts = [None] * BLK

            def t_unit(b4):
                if not pb4:
                    return
                pv = psum_pv.tile(
                    [128, 4, 256], f32, tag="pv", name=f"pv_{pb4[0]}_{b4}"
                )
                tail_pv_g(pv, pb4[1], pb4[2], b4, 0)
                tail_pv_g(pv, pb4[1], pb4[2], b4, 1)
                xnts[b4] = tail_norm(pv, pb4[0], b4)

            qk_exp(0, 0)
            qk_exp(0, 1)
            t_unit(0)
            mul(0, 0)
            qk_exp(0, 2)
            t_unit(1)
            qk_exp(0, 3)
            mul(0, 1)
            t_unit(2)
            qk_exp(1, 0)
            t_unit(3)
            qk_exp(1, 1)
            mul(1, 0)
            if pb4:
                tail_proj(pb4[0], 0, xnts[0])
            qk_exp(1, 2)
            if pb4:
                tail_proj(pb4[0], 1, xnts[1])
                tail_proj(pb4[0], 2, xnts[2])
            qk_exp(1, 3)
            mul(1, 1)
            if pb4:
                tail_proj(pb4[0], 3, xnts[3])

            prev = (bb, v_sb, pst)

        for b4 in range(BLK):
            pv = psum_pv.tile([128, 4, 256], f32, tag="pv", name=f"pvf_{b4}")
            tail_pv_g(pv, prev[1], prev[2], b4, 0)
            tail_pv_g(pv, prev[1], prev[2], b4, 1)
            xnt = tail_norm(pv, prev[0], b4)
            tail_proj(prev[0], b4, xnt)

    nc.compile()
    _CACHED["nc"] = nc
    return nc


def _prep_host(q, k, v, dpb_w1, dpb_b1, dpb_w2, dpb_b2, proj_w, proj_b):
    scale = HD ** -0.5
    # qT/kT [2, 128, B, 196]: [g, 32*l + d, b, n] = q[b, n, (4g+l)*32 + d]
    qs = (q.astype(np.float32) * scale).transpose(2, 0, 1).reshape(2, 128, B, N)
    qt = np.ascontiguousarray(qs).astype(BF16)
    ks = k.astype(np.float32).transpose(2, 0, 1).reshape(2, 128, B, N)
    kt = np.ascontiguousarray(ks).astype(BF16)
    # v [98, B, 2, 8, 32]: [p, b, j, h, d] = v[b, 98j+p, 32h+d]
    vr = v.reshape(B, 2, NC2, NH, HD).transpose(2, 0, 1, 3, 4)
    vx = np.ascontiguousarray(vr).astype(BF16)
    # rpb via MLP on host
    biases = _bias_coords(G)
    pos = np.maximum(biases @ dpb_w1 + dpb_b1, 0.0) @ dpb_w2 + dpb_b2  # [729, 8]
    idx = _relative_position_index(G).reshape(-1)
    rpb = pos[idx].reshape(N, N, NH).transpose(2, 0, 1)  # [H, n, m]
    # erpb [98, 2, 2, 4, 2, 196]: [p, g, brep, l, j, n] = exp(rpb[4g+l, n, 98j+p])
    # (replicated over a 2-wide b axis so the DVE multiply reads contiguous
    # stride-1 data and hits the 2x bf16 perf mode)
    er = np.exp(rpb)  # [h, n, m]
    erpb1 = np.empty((NC2, 2, 4, 2, N), np.float32)
    for g in range(2):
        for l in range(4):
            for j in range(2):
                erpb1[:, g, l, j, :] = er[4 * g + l, :, 98 * j : 98 * j + 98].T
    erpb = np.ascontiguousarray(
        np.broadcast_to(erpb1[:, :, None], (NC2, 2, 2, 4, 2, N))
    ).astype(BF16)
    # w [128, 2, 256]: [32l+d, g, co] = proj_w[(4g+l)*32 + d, co]
    w = np.ascontiguousarray(
        proj_w.reshape(2, 128, 256).transpose(1, 0, 2)
    ).astype(BF16)
    pb = np.broadcast_to(proj_b.reshape(1, 256), (NC2, 256))
    pb = np.ascontiguousarray(pb).astype(BF16)
    return qt, kt, vx, erpb, w, pb


def kernel(**inputs) -> np.ndarray:
    q = np.asarray(inputs["q"], np.float32)
    k = np.asarray(inputs["k"], np.float32)
    v = np.asarray(inputs["v"], np.float32)
    qt, kt, vx, erpb, w, pb = _prep_host(
        q, k, v,
        np.asarray(inputs["dpb_w1"], np.float32),
        np.asarray(inputs["dpb_b1"], np.float32),
        np.asarray(inputs["dpb_w2"], np.float32),
        np.asarray(inputs["dpb_b2"], np.float32),
        np.asarray(inputs["proj_w"], np.float32),
        np.asarray(inputs["proj_b"], np.float32),
    )
    nc = _build_bass()
    in_maps = []
    for c in range(NCORES):
        sl = slice(c * BLOC, (c + 1) * BLOC)
        in_maps.append(
            {
                "qt": np.ascontiguousarray(qt[:, :, sl]),
                "kt": np.ascontiguousarray(kt[:, :, sl]),
                "v": np.ascontiguousarray(vx[:, sl]),
                "erpb": erpb,
                "w": w,
                "pb": pb,
            }
        )
    res = run_bass_kernel_spmd(
        nc, in_maps, core_ids=list(range(NCORES)), trace=bool(_CACHED.get("trace"))
    )
    _CACHED["last_results"] = res
    out = np.concatenate([r["out"] for r in res.results], axis=0)
    return out.astype(np.float32)


if __name__ == "__main__":
    rng = np.random.default_rng(0)
    ins = {
        "q": rng.standard_normal((B, N, DIM), dtype=np.float32),
        "k": rng.standard_normal((B, N, DIM), dtype=np.float32),
        "v": rng.standard_normal((B, N, DIM), dtype=np.float32),
        "dpb_w1": rng.standard_normal((2, 64), dtype=np.float32) * 0.1,
        "dpb_b1": np.zeros(64, np.float32),
        "dpb_w2": rng.standard_normal((64, 8), dtype=np.float32) * 0.1,
        "dpb_b2": np.zeros(8, np.float32),
        "proj_w": rng.standard_normal((256, 256), dtype=np.float32) * (256 ** -0.5),
        "proj_b": np.zeros(256, np.float32),
        "group_size": 14,
    }
    o = kernel(**ins)
    print(o.shape, o.dtype)


# revision 45
# speedup vs baseline: 1.0145x; 1.0030x over previous
import sys

for p in ("/opt/trn_rl_repo",):
    if p not in sys.path:
        sys.path.insert(0, p)

import numpy as np
import ml_dtypes

import concourse.bass as bass
from concourse import bacc
import concourse.mybir as mybir
import concourse.tile as tile
from concourse.bass import ds, ts
from concourse.bass_utils import run_bass_kernel_spmd

BF16 = ml_dtypes.bfloat16

B, N, DIM, NH = 256, 196, 256, 8
HD = DIM // NH  # 32
G = 14
NCORES = 8
BLOC = B // NCORES  # 32
NC2 = 98  # N / 2
BLK = 4  # batch block per pipeline stage

# QK psum supertile [98, 4, 2, 256] f32 = 4 banks per (b, group): local head l
# owns bank l (so each PE row-tile writes exactly one bank — row tiles must
# never share a bank), with the two m-chunks j at 1 KiB offsets inside it.


def _relative_position_index(g: int) -> np.ndarray:
    coords = np.stack(np.meshgrid(np.arange(g), np.arange(g), indexing="ij"))
    cf = coords.reshape(2, -1)
    rel = cf[:, :, None] - cf[:, None, :]
    rel = rel.transpose(1, 2, 0).astype(np.int64)
    rel[..., 0] += g - 1
    rel[..., 1] += g - 1
    rel[..., 0] *= 2 * g - 1
    return rel.sum(-1)


def _bias_coords(g: int) -> np.ndarray:
    p = np.arange(1 - g, g)
    biases = np.stack(np.meshgrid(p, p, indexing="ij"))
    return biases.reshape(2, -1).T.astype(np.float32)


_CACHED = {}


def _build_bass():
    if "nc" in _CACHED:
        return _CACHED["nc"]
    f32 = mybir.dt.float32
    bf16 = mybir.dt.bfloat16

    nc = bacc.Bacc("TRN2", target_bir_lowering=False)
    qt_d = nc.dram_tensor("qt", [2, 128, BLOC, 196], bf16, kind="ExternalInput")
    kt_d = nc.dram_tensor("kt", [2, 128, BLOC, 196], bf16, kind="ExternalInput")
    v_d = nc.dram_tensor("v", [NC2, BLOC, 2, 8, 32], bf16, kind="ExternalInput")
    erpb_d = nc.dram_tensor("erpb", [NC2, 2, 2, 4, 2, 196], bf16, kind="ExternalInput")
    w_d = nc.dram_tensor("w", [128, 2, 256], bf16, kind="ExternalInput")
    pb_d = nc.dram_tensor("pb", [NC2, 256], bf16, kind="ExternalInput")
    out_d = nc.dram_tensor("out", [BLOC, 196, 256], bf16, kind="ExternalOutput")

    from contextlib import ExitStack

    with tile.TileContext(nc) as tc, ExitStack() as es:
        const = es.enter_context(tc.tile_pool(name="const", bufs=1))
        io = es.enter_context(tc.tile_pool(name="io", bufs=2))
        work = es.enter_context(tc.tile_pool(name="work", bufs=2))
        psum_qk = es.enter_context(tc.tile_pool(name="psum_qk", bufs=2, space="PSUM"))
        psum_pv = es.enter_context(tc.tile_pool(name="psum_pv", bufs=1, space="PSUM"))
        psum_po = es.enter_context(tc.tile_pool(name="psum_po", bufs=2, space="PSUM"))

        erpb_sb = const.tile([NC2, 2, 2, 4, 2, 196], bf16)
        nc.sync.dma_start(erpb_sb[:], erpb_d[:])
        w_sb = const.tile([128, 2, 256], bf16)
        nc.sync.dma_start(w_sb[:], w_d[:])
        pb_sb = const.tile([NC2, 256], bf16)
        nc.sync.dma_start(pb_sb[:], pb_d[:])
        ones32 = const.tile([NC2, 32], bf16)
        nc.vector.memset(ones32[:], 1.0)
        # preload the exp table set during the initial DMA wait
        warm = const.tile([1, 8], f32)
        nc.scalar.activation(warm[:], warm[:], mybir.ActivationFunctionType.Exp)

        def tail_pv_g(pv, v_sb, pst, b4, g):
            # pv slots: [xT_g0, xT_g1, den_g0, den_g1] → xT in bank 0,
            # denominators (replicated via 32-wide ones weights) in bank 1
            for l in range(4):
                for j in range(2):
                    nc.tensor.matmul(
                        pv[ds(32 * l, 32), g, 0:196],
                        lhsT=v_sb[:, b4, j, 4 * g + l],
                        rhs=pst[g][:, b4, l, j],
                        start=(j == 0),
                        stop=(j == 1),
                        tile_position=(0, 32 * l),
                    )
            for l in range(4):
                for j in range(2):
                    nc.tensor.matmul(
                        pv[ds(32 * l, 32), 2 + g, 0:196],
                        lhsT=ones32[:],
                        rhs=pst[g][:, b4, l, j],
                        start=(j == 0),
                        stop=(j == 1),
                        tile_position=(0, 32 * l),
                    )

        def tail_norm(pv, bb, b4):
            rcp = work.tile([128, 2, 196], f32, tag="rcp", name=f"rcp_{bb}_{b4}")
            nc.vector.reciprocal_approx_fast(rcp[:], pv[:, 2:4, 0:196])
            xnt = work.tile(
                [128, 2, 196], bf16, tag="xnt", name=f"xnt_{bb}_{b4}", bufs=4
            )
            nc.vector.tensor_mul(out=xnt[:], in0=pv[:, 0:2, 0:196], in1=rcp[:])
            return xnt

        def tail_proj(bb, b4, xnt):
            po = psum_po.tile([NC2, 2, 256], f32, tag="po", name=f"po_{bb}_{b4}")
            for i in range(2):
                for g in range(2):
                    nc.tensor.matmul(
                        po[:, i],
                        lhsT=xnt[:, g, ds(98 * i, 98)],
                        rhs=w_sb[:, g],
                        start=(g == 0),
                        stop=(g == 1),
                    )
            o_sb = work.tile([NC2, 2, 256], bf16, tag="o", name=f"o_{bb}_{b4}")
            nc.vector.tensor_add(
                out=o_sb[:],
                in0=po[:],
                in1=pb_sb[:, None].to_broadcast([NC2, 2, 256]),
            )
            nc.sync.dma_start(
                out_d[bb + b4].rearrange("(i p) c -> p i c", p=NC2), o_sb[:]
            )

        def load_block(bb):
            qt_sb = [
                io.tile([128, BLK, 196], bf16, tag=f"qt{g}", name=f"qt{g}_{bb}")
                for g in range(2)
            ]
            kt_sb = [
                io.tile([128, BLK, 196], bf16, tag=f"kt{g}", name=f"kt{g}_{bb}")
                for g in range(2)
            ]
            for g in range(2):
                nc.sync.dma_start(qt_sb[g][:], qt_d[g, :, ds(bb, BLK)])
                nc.sync.dma_start(kt_sb[g][:], kt_d[g, :, ds(bb, BLK)])
            v_sb = io.tile(
                [NC2, BLK, 2, 8, 32], bf16, tag="v", name=f"v_{bb}", bufs=3
            )
            nc.sync.dma_start(v_sb[:], v_d[:, ds(bb, BLK)])
            return qt_sb, kt_sb, v_sb

        prev = None
        loaded = load_block(0)
        for bb in range(0, BLOC, BLK):
            qt_sb, kt_sb, v_sb = loaded
            if bb + BLK < BLOC:
                loaded = load_block(bb + BLK)

            est = {}
            pst = {}
            for g in range(2):
                est[g] = work.tile(
                    [NC2, BLK, 4, 2, 196], bf16, tag=f"est{g}", name=f"est{g}_{bb}"
                )
                pst[g] = work.tile(
                    [NC2, BLK, 4, 2, 196], bf16, tag=f"pst{g}", name=f"pst{g}_{bb}"
                )

            def qk_exp(g, b4):
                # per (b, g, head-pair) supertile [98, 2, 2, 256] f32 = 2
                # banks: each head's PE row-tile owns one full psum bank;
                # 2-bank halves double-buffer so ACT never waits on refill
                for hp in range(2):
                    sqk = psum_qk.tile(
                        [NC2, 2, 2, 256], f32, tag="qk", name=f"qk{g}{hp}_{bb}_{b4}"
                    )
                    for j in range(2):
                        for dl in range(2):
                            l = 2 * hp + dl
                            nc.tensor.matmul(
                                sqk[:, dl, j, 0:196],
                                lhsT=kt_sb[g][ds(32 * l, 32), b4, ds(98 * j, 98)],
                                rhs=qt_sb[g][ds(32 * l, 32), b4],
                                start=True,
                                stop=True,
                                tile_position=(32 * l, 0),
                            )
                    nc.scalar.activation(
                        est[g][:, b4, ds(2 * hp, 2)],
                        sqk[:, :, :, 0:196],
                        mybir.ActivationFunctionType.Exp,
                    )

            def mul(g, h2):
                nc.vector.tensor_mul(
                    out=pst[g][:, ds(2 * h2, 2)],
                    in0=est[g][:, ds(2 * h2, 2)],
                    in1=erpb_sb[:, g],
                )

            # software-pipelined issue: exp stages of block i interleaved with
            # PV/norm of block i-1 and late proj units of block i-2, so the
            # PE FIFO never holds ACT hostage at block seams
            pb4 = prev
            xnts = [None] * BLK

            def t_unit(b4):
                if not pb4:
                    return
                pv = psum_pv.tile(
                    [128, 4, 256], f32, tag="pv", name=f"pv_{pb4[0]}_{b4}"
                )
                tail_pv_g(pv, pb4[1], pb4[2], b4, 0)
                tail_pv_g(pv, pb4[1], pb4[2], b4, 1)
                xnts[b4] = tail_norm(pv, pb4[0], b4)

            qk_exp(0, 0)
            qk_exp(0, 1)
            t_unit(0)
            qk_exp(0, 2)
            t_unit(1)
            mul(0, 0)
            qk_exp(0, 3)
            t_unit(2)
            mul(0, 1)
            qk_exp(1, 0)
            t_unit(3)
            qk_exp(1, 1)
            mul(1, 0)
            if pb4:
                tail_proj(pb4[0], 0, xnts[0])
            qk_exp(1, 2)
            if pb4:
                tail_proj(pb4[0], 1, xnts[1])
                tail_proj(pb4[0], 2, xnts[2])
            qk_exp(1, 3)
            mul(1, 1)
            if pb4:
                tail_proj(pb4[0], 3, xnts[3])

            prev = (bb, v_sb, pst)

        for b4 in range(BLK):
            pv = psum_pv.tile([128, 4, 256], f32, tag="pv", name=f"pvf_{b4}")
            tail_pv_g(pv, prev[1], prev[2], b4, 0)
            tail_pv_g(pv, prev[1], prev[2], b4, 1)
            xnt = tail_norm(pv, prev[0], b4)
            tail_proj(prev[0], b4, xnt)

    nc.compile()
    _CACHED["nc"] = nc
    return nc


def _prep_host(q, k, v, dpb_w1, dpb_b1, dpb_w2, dpb_b2, proj_w, proj_b):
    scale = HD ** -0.5
    # qT/kT [2, 128, B, 196]: [g, 32*l + d, b, n] = q[b, n, (4g+l)*32 + d]
    qs = (q.astype(np.float32) * scale).transpose(2, 0, 1).reshape(2, 128, B, N)
    qt = np.ascontiguousarray(qs).astype(BF16)
    ks = k.astype(np.float32).transpose(2, 0, 1).reshape(2, 128, B, N)
    kt = np.ascontiguousarray(ks).astype(BF16)
    # v [98, B, 2, 8, 32]: [p, b, j, h, d] = v[b, 98j+p, 32h+d]
    vr = v.reshape(B, 2, NC2, NH, HD).transpose(2, 0, 1, 3, 4)
    vx = np.ascontiguousarray(vr).astype(BF16)
    # rpb via MLP on host
    biases = _bias_coords(G)
    pos = np.maximum(biases @ dpb_w1 + dpb_b1, 0.0) @ dpb_w2 + dpb_b2  # [729, 8]
    idx = _relative_position_index(G).reshape(-1)
    rpb = pos[idx].reshape(N, N, NH).transpose(2, 0, 1)  # [H, n, m]
    # erpb [98, 2, 2, 4, 2, 196]: [p, g, brep, l, j, n] = exp(rpb[4g+l, n, 98j+p])
    # (replicated over a 2-wide b axis so the DVE multiply reads contiguous
    # stride-1 data and hits the 2x bf16 perf mode)
    er = np.exp(rpb)  # [h, n, m]
    erpb1 = np.empty((NC2, 2, 4, 2, N), np.float32)
    for g in range(2):
        for l in range(4):
            for j in range(2):
                erpb1[:, g, l, j, :] = er[4 * g + l, :, 98 * j : 98 * j + 98].T
    erpb = np.ascontiguousarray(
        np.broadcast_to(erpb1[:, :, None], (NC2, 2, 2, 4, 2, N))
    ).astype(BF16)
    # w [128, 2, 256]: [32l+d, g, co] = proj_w[(4g+l)*32 + d, co]
    w = np.ascontiguousarray(
        proj_w.reshape(2, 128, 256).transpose(1, 0, 2)
    ).astype(BF16)
    pb = np.broadcast_to(proj_b.reshape(1, 256), (NC2, 256))
    pb = np.ascontiguousarray(pb).astype(BF16)
    return qt, kt, vx, erpb, w, pb


def kernel(**inputs) -> np.ndarray:
    q = np.asarray(inputs["q"], np.float32)
    k = np.asarray(inputs["k"], np.float32)
    v = np.asarray(inputs["v"], np.float32)
    qt, kt, vx, erpb, w, pb = _prep_host(
        q, k, v,
        np.asarray(inputs["dpb_w1"], np.float32),
        np.asarray(inputs["dpb_b1"], np.float32),
        np.asarray(inputs["dpb_w2"], np.float32),
        np.asarray(inputs["dpb_b2"], np.float32),
        np.asarray(inputs["proj_w"], np.float32),
        np.asarray(inputs["proj_b"], np.float32),
    )
    nc = _build_bass()
    in_maps = []
    for c in range(NCORES):
        sl = slice(c * BLOC, (c + 1) * BLOC)
        in_maps.append(
            {
                "qt": np.ascontiguousarray(qt[:, :, sl]),
                "kt": np.ascontiguousarray(kt[:, :, sl]),
                "v": np.ascontiguousarray(vx[:, sl]),
                "erpb": erpb,
                "w": w,
                "pb": pb,
            }
        )
    res = run_bass_kernel_spmd(
        nc, in_maps, core_ids=list(range(NCORES)), trace=bool(_CACHED.get("trace"))
    )
    _CACHED["last_results"] = res
    out = np.concatenate([r["out"] for r in res.results], axis=0)
    return out.astype(np.float32)


if __name__ == "__main__":
    rng = np.random.default_rng(0)
    ins = {
        "q": rng.standard_normal((B, N, DIM), dtype=np.float32),
        "k": rng.standard_normal((B, N, DIM), dtype=np.float32),
        "v": rng.standard_normal((B, N, DIM), dtype=np.float32),
        "dpb_w1": rng.standard_normal((2, 64), dtype=np.float32) * 0.1,
        "dpb_b1": np.zeros(64, np.float32),
        "dpb_w2": rng.standard_normal((64, 8), dtype=np.float32) * 0.1,
        "dpb_b2": np.zeros(8, np.float32),
        "proj_w": rng.standard_normal((256, 256), dtype=np.float32) * (256 ** -0.5),
        "proj_b": np.zeros(256, np.float32),
        "group_size": 14,
    }
    o = kernel(**ins)
    print(o.shape, o.dtype)
